# revision 6
# baseline (speedup 1.0000x reference)
"""Trainium2 Bass kernel for CachedMixtralAttention (sliding-window GQA attention).

Strategy (8 NeuronCores, tensor-parallel over KV-head groups):
  - Core i handles KV head i and its 4 query heads (GQA group). Wq/Wk/Wv are
    sliced on the head axis, Wo on the input-head axis. Each core computes a
    partial output [S, HID]; the host sums the 8 partials.
  - Host passes hidden^T (pre-transposed) so every matmul operand streams from
    DRAM in its natural layout; no on-device transposes of the activations.
  - On-device layout is "T layout": QT/KT = [head_dim, seq] so the attention
    contraction dims always sit on SBUF partitions.
  - Softmax skips the max-subtraction (scores ~ N(0,1) after 1/sqrt(d): exp is
    safe in fp32) and applies the mask as a 0/1 multiply after exp, which is
    exactly equivalent to the reference's -1e9 masking.
  - Mask handling is data-driven: each [128 k, 512 q] block of the mask is
    classified on host as skip / full / partial; only partial blocks pay a
    multiply, all-False blocks are never computed. A sliding window of 512
    yields a ~1024-wide band per 512 queries.
  - matmul inputs in bf16, PSUM accumulation + softmax math in fp32.
"""

from contextlib import ExitStack

import ml_dtypes
import numpy as np

S = 2048
HID = 4096
NUM_Q_HEADS = 32
NUM_KV_HEADS = 8
D = 128                      # head dim
NCORES = 8
HQ = NUM_Q_HEADS // NUM_KV_HEADS  # q heads per core (GQA group size)
QC = 512                     # query chunk (matmul moving free dim)
MAX_WAVELENGTH = 10000.0
INV_NORM = 1.0 / np.sqrt(D)

BF16 = ml_dtypes.bfloat16


def _rope_tables(s):
    """cos/sin tables in T layout [128, s], sign folded into sin."""
    pos = np.arange(s, dtype=np.float32)
    invf = 1.0 / (MAX_WAVELENGTH ** (np.arange(0, D, 2, dtype=np.float32) / D))
    freq = invf[:, None] * pos[None, :]              # [64, s]
    cosT = np.concatenate([np.cos(freq), np.cos(freq)], axis=0)   # [128, s]
    sinT = np.concatenate([-np.sin(freq), np.sin(freq)], axis=0)  # [128, s]
    return cosT.astype(np.float32), sinT.astype(np.float32)


def _classify_mask(mask2d, s):
    """Classify [128k x QCq] blocks of the mask: skip / full / partial.

    Returns (blocks, mask_tiles): blocks[c] is a list of (g, mask_id) with
    g the global k-tile index and mask_id None for full blocks; mask_tiles
    is [n, 128, QC] float32 of the partial blocks (n >= 1, padded).
    """
    mT = np.ascontiguousarray(mask2d.T)  # [k, q]
    n_chunks = s // QC
    n_ktiles = s // 128
    blocks = []
    tiles = []
    tile_ids = {}
    for c in range(n_chunks):
        lst = []
        for g in range(n_ktiles):
            blk = mT[g * 128:(g + 1) * 128, c * QC:(c + 1) * QC]
            if not blk.any():
                continue
            if blk.all():
                lst.append((g, None))
            else:
                key = blk.tobytes()
                if key not in tile_ids:
                    tile_ids[key] = len(tiles)
                    tiles.append(blk.astype(np.float32))
                lst.append((g, tile_ids[key]))
        assert lst, f"query chunk {c} attends to nothing"
        blocks.append(lst)
    if not tiles:
        tiles.append(np.zeros((128, QC), np.float32))
    return blocks, np.stack(tiles)


def _build_program(s, hid, blocks, n_mask):
    """Emit the Bass/Tile program. Same program runs SPMD on all 8 cores."""
    import concourse.bacc as bacc
    import concourse.mybir as mybir
    import concourse.tile as tile

    dt = mybir.dt
    HT = hid // 128          # hidden contraction tiles
    C = s // QC              # query chunks
    ST = s // 128            # seq tiles of 128

    nc = bacc.Bacc("TRN2", target_bir_lowering=False, debug=False,
                   num_devices=NCORES)

    hT_d = nc.declare_dram_parameter("hT", [hid, s], dt.bfloat16, isOutput=False)
    wq_d = nc.declare_dram_parameter("wq", [hid, HQ * D], dt.bfloat16, isOutput=False)
    wk_d = nc.declare_dram_parameter("wk", [hid, D], dt.bfloat16, isOutput=False)
    wv_d = nc.declare_dram_parameter("wv", [hid, D], dt.bfloat16, isOutput=False)
    wo_d = nc.declare_dram_parameter("wo", [HQ * D, hid], dt.bfloat16, isOutput=False)
    cos_d = nc.declare_dram_parameter("cosT", [128, s], dt.float32, isOutput=False)
    sin_d = nc.declare_dram_parameter("sinT", [128, s], dt.float32, isOutput=False)
    msk_d = nc.declare_dram_parameter("masks", [n_mask, 128, QC], dt.bfloat16, isOutput=False)
    eye_d = nc.declare_dram_parameter("eye", [128, 128], dt.bfloat16, isOutput=False)
    out_d = nc.declare_dram_parameter("out", [s, hid], dt.float32, isOutput=True)

    with ExitStack() as ctx:
        tc = ctx.enter_context(tile.TileContext(nc))
        const = ctx.enter_context(tc.tile_pool(name="const", bufs=1))
        hpool = ctx.enter_context(tc.tile_pool(name="hpool", bufs=4))
        epool = ctx.enter_context(tc.tile_pool(name="epool", bufs=4))
        tpool = ctx.enter_context(tc.tile_pool(name="tpool", bufs=3))
        opool = ctx.enter_context(tc.tile_pool(name="opool", bufs=3))
        psum = ctx.enter_context(tc.tile_pool(name="psum", bufs=8, space="PSUM"))

        # ---- one-time loads (weights resident in SBUF) ----
        wq_sb = const.tile([128, HT * HQ * D], dt.bfloat16, tag="wq")
        nc.sync.dma_start(wq_sb[:].rearrange("p (t c) -> p t c", t=HT),
                          wq_d[:].rearrange("(t p) c -> p t c", p=128))
        wk_sb = const.tile([128, HT * D], dt.bfloat16, tag="wk")
        nc.sync.dma_start(wk_sb[:].rearrange("p (t c) -> p t c", t=HT),
                          wk_d[:].rearrange("(t p) c -> p t c", p=128))
        wv_sb = const.tile([128, HT * D], dt.bfloat16, tag="wv")
        nc.sync.dma_start(wv_sb[:].rearrange("p (t c) -> p t c", t=HT),
                          wv_d[:].rearrange("(t p) c -> p t c", p=128))
        wo_sb = const.tile([128, HQ * hid], dt.bfloat16, tag="wo")
        nc.sync.dma_start(wo_sb[:].rearrange("p (j o) -> p j o", j=HQ),
                          wo_d[:].rearrange("(j p) o -> p j o", p=128))
        msk_sb = const.tile([128, n_mask * QC], dt.bfloat16, tag="msk")
        nc.sync.dma_start(msk_sb[:].rearrange("p (m q) -> p m q", m=n_mask),
                          msk_d[:].rearrange("m p q -> p m q"))
        eye_sb = const.tile([128, 128], dt.bfloat16, tag="eye")
        nc.sync.dma_start(eye_sb[:], eye_d[:])
        ones_sb = const.tile([128, 1], dt.bfloat16, tag="ones")
        nc.vector.memset(ones_sb[:], 1.0)
        ones_b_sb = const.tile([1, 128], dt.bfloat16, tag="ones_b")
        nc.vector.memset(ones_b_sb[:], 1.0)

        # persistent per-chunk tensors
        q_sb = [[const.tile([128, QC], dt.bfloat16, tag=f"q{c}_{h}", name=f"q{c}_{h}")
                 for h in range(HQ)] for c in range(C)]
        kt_sb = [const.tile([128, QC], dt.bfloat16, tag=f"kt{c}", name=f"kt{c}")
                 for c in range(C)]
        v_sb = [[const.tile([128, 128], dt.bfloat16, tag=f"v{c}_{j}", name=f"v{c}_{j}")
                 for j in range(QC // 128)] for c in range(C)]
        at_sb = [[const.tile([128, QC], dt.bfloat16, tag=f"at{c}_{h}", name=f"at{c}_{h}")
                  for h in range(HQ)] for c in range(C)]

        # ---- phase 1: QKV projections (T layout) + RoPE + V transpose ----
        def rope(ps, dest, cos_sb, sin_sb):
            a = tpool.tile([128, QC], dt.float32, bufs=3, name="a")
            nc.vector.tensor_copy(a[:], ps[:])
            b = tpool.tile([128, QC], dt.float32, bufs=3, name="b")
            nc.sync.dma_start(b[0:64, :], a[64:128, :])
            nc.sync.dma_start(b[64:128, :], a[0:64, :])
            t1 = tpool.tile([128, QC], dt.float32, name="t1")
            nc.vector.tensor_mul(t1[:], a[:], cos_sb[:])
            t2 = tpool.tile([128, QC], dt.float32, name="t2")
            nc.vector.tensor_mul(t2[:], b[:], sin_sb[:])
            nc.vector.tensor_add(dest[:], t1[:], t2[:])

        for c in range(C):
            cos_sb = tpool.tile([128, QC], dt.float32, tag="cosc", bufs=2)
            nc.sync.dma_start(cos_sb[:], cos_d[:, c * QC:(c + 1) * QC])
            sin_sb = tpool.tile([128, QC], dt.float32, tag="sinc", bufs=2)
            nc.sync.dma_start(sin_sb[:], sin_d[:, c * QC:(c + 1) * QC])
            # wave A: q heads 0,1 + K;  wave B: q heads 2,3 + V
            # (3 PSUM accumulators per wave keeps chunk boundaries fluid)
            for wave in range(2):
                acc = [psum.tile([128, QC], dt.float32, name=f"acc{w}", tag="ps")
                       for w in range(3)]
                if wave == 0:
                    cols = [HQ * 0, HQ * 0 + 1, None]   # q0, q1, K
                else:
                    cols = [2, 3, None]                  # q2, q3, V
                for t in range(HT):
                    ht = hpool.tile([128, QC], dt.bfloat16)
                    nc.sync.dma_start(ht[:], hT_d[t * 128:(t + 1) * 128,
                                                  c * QC:(c + 1) * QC])
                    st, sp = (t == 0), (t == HT - 1)
                    for w in range(2):
                        h = cols[w]
                        nc.tensor.matmul(acc[w][:],
                                         wq_sb[:, t * HQ * D + h * D: t * HQ * D + (h + 1) * D],
                                         ht[:], start=st, stop=sp)
                    wkv = wk_sb if wave == 0 else wv_sb
                    nc.tensor.matmul(acc[2][:], wkv[:, t * D:(t + 1) * D], ht[:],
                                     start=st, stop=sp)
                if wave == 0:
                    rope(acc[0], q_sb[c][0], cos_sb, sin_sb)
                    rope(acc[1], q_sb[c][1], cos_sb, sin_sb)
                    rope(acc[2], kt_sb[c], cos_sb, sin_sb)
                else:
                    rope(acc[0], q_sb[c][2], cos_sb, sin_sb)
                    rope(acc[1], q_sb[c][3], cos_sb, sin_sb)
                    # V: VT [d, k] -> transpose 128x128 blocks -> V [k, d]
                    vtT = epool.tile([128, QC], dt.bfloat16, bufs=2)
                    nc.scalar.copy(vtT[:], acc[2][:])
                    for j in range(QC // 128):
                        tp = psum.tile([128, 128], dt.bfloat16, tag="ps")
                        nc.tensor.transpose(tp[:], vtT[:, j * 128:(j + 1) * 128],
                                            eye_sb[:])
                        nc.vector.tensor_copy(v_sb[c][j][:], tp[:])

        # ---- phase 2: attention per (chunk, head) ----
        # Normalization of head h is emitted after head h+1's score stream so
        # the in-order PE queue never stalls on the DVE reciprocal.
        def emit_norm(c, h, at_ps, den_ps):
            rec = tpool.tile([1, QC], dt.float32, tag="rec", bufs=3, name="rec")
            nc.vector.reciprocal(rec[:], den_ps[0:1, :])
            recb = tpool.tile([1, QC], dt.bfloat16, tag="recb", bufs=3, name="recb")
            nc.vector.tensor_copy(recb[:], rec[:])
            bc = psum.tile([128, QC], dt.float32, tag="ps", name="bc")
            nc.tensor.matmul(bc[:], ones_b_sb[:], recb[:], start=True, stop=True)
            bc_sb = tpool.tile([128, QC], dt.float32, tag="bcs", bufs=3, name="bc_sb")
            nc.vector.tensor_copy(bc_sb[:], bc[:])
            nc.vector.tensor_mul(at_sb[c][h][:], at_ps[:], bc_sb[:])

        pend = None
        for c in range(C):
            blks = blocks[c]
            for h in range(HQ):
                at_ps = psum.tile([128, QC], dt.float32, tag="ps")
                den_ps = psum.tile([128, QC], dt.float32, tag="ps")
                for i, (g, mid) in enumerate(blks):
                    kc, j = g // (QC // 128), g % (QC // 128)
                    sc = psum.tile([128, QC], dt.float32, tag="ps")
                    nc.tensor.matmul(sc[:], kt_sb[kc][:, j * 128:(j + 1) * 128],
                                     q_sb[c][h][:], start=True, stop=True)
                    e = epool.tile([128, QC], dt.bfloat16)
                    nc.scalar.activation(e[:], sc[:],
                                         mybir.ActivationFunctionType.Exp,
                                         scale=float(INV_NORM))
                    if mid is not None:
                        nc.vector.tensor_mul(e[:], e[:],
                                             msk_sb[:, mid * QC:(mid + 1) * QC])
                    st, sp = (i == 0), (i == len(blks) - 1)
                    nc.tensor.matmul(at_ps[:], v_sb[kc][j][:], e[:], start=st, stop=sp)
                    nc.tensor.matmul(den_ps[0:1, :], ones_sb[:], e[:], start=st, stop=sp)
                if pend is not None:
                    emit_norm(*pend)
                pend = (c, h, at_ps, den_ps)
        emit_norm(*pend)

        # ---- phase 3: output projection ----
        for oc in range(hid // QC):
            for sti in range(ST):
                c, r = sti // (QC // 128), sti % (QC // 128)
                o_ps = psum.tile([128, QC], dt.float32, tag="ps")
                for h in range(HQ):
                    nc.tensor.matmul(o_ps[:],
                                     at_sb[c][h][:, r * 128:(r + 1) * 128],
                                     wo_sb[:, h * hid + oc * QC: h * hid + (oc + 1) * QC],
                                     start=(h == 0), stop=(h == HQ - 1))
                o_sb = opool.tile([128, QC], dt.float32)
                nc.vector.tensor_copy(o_sb[:], o_ps[:])
                nc.sync.dma_start(out_d[sti * 128:(sti + 1) * 128, oc * QC:(oc + 1) * QC],
                                  o_sb[:])

    nc.compile()
    return nc


def _prep_inputs(hidden_states, attention_mask, Wq, Wk, Wv, Wo):
    """Host-side sharding + layout prep. Returns (in_maps, blocks, n_mask, s, hid)."""
    hs = np.asarray(hidden_states)
    assert hs.shape[0] == 1, "kernel assumes batch 1"
    s, hid = hs.shape[1], hs.shape[2]
    mask = np.asarray(attention_mask)[0]
    Wq = np.asarray(Wq); Wk = np.asarray(Wk); Wv = np.asarray(Wv); Wo = np.asarray(Wo)

    hT = np.ascontiguousarray(hs[0].T).astype(BF16)          # [hid, s]
    cosT, sinT = _rope_tables(s)
    blocks, mask_tiles = _classify_mask(mask, s)
    masks_bf = mask_tiles.astype(BF16)
    eye = np.eye(128, dtype=np.float32).astype(BF16)

    in_maps = []
    for i in range(NCORES):
        wq_i = np.ascontiguousarray(
            Wq[:, i * HQ:(i + 1) * HQ, :].reshape(hid, HQ * D)).astype(BF16)
        wk_i = np.ascontiguousarray(Wk[:, i, :]).astype(BF16)
        wv_i = np.ascontiguousarray(Wv[:, i, :]).astype(BF16)
        wo_i = np.ascontiguousarray(
            Wo[i * HQ:(i + 1) * HQ].reshape(HQ * D, hid)).astype(BF16)
        in_maps.append({
            "hT": hT, "wq": wq_i, "wk": wk_i, "wv": wv_i, "wo": wo_i,
            "cosT": cosT, "sinT": sinT, "masks": masks_bf, "eye": eye,
        })
    return in_maps, blocks, masks_bf.shape[0], s, hid


def _run(hidden_states, attention_mask, Wq, Wk, Wv, Wo, trace=False):
    from concourse.bass_utils import run_bass_kernel_spmd

    in_maps, blocks, n_mask, s, hid = _prep_inputs(
        hidden_states, attention_mask, Wq, Wk, Wv, Wo)
    nc = _build_program(s, hid, blocks, n_mask)
    res = run_bass_kernel_spmd(nc, in_maps, core_ids=list(range(NCORES)),
                               trace=trace)
    parts = [res.results[i]["out"].astype(np.float32) for i in range(NCORES)]
    out = parts[0]
    for p in parts[1:]:
        out = out + p
    return out[None, :, :], res


def kernel(hidden_states, attention_mask, Wq, Wk, Wv, Wo):
    out, _ = _run(hidden_states, attention_mask, Wq, Wk, Wv, Wo, trace=False)
    return out
